# revision 4
# baseline (speedup 1.0000x reference)
"""Trainium2 Bass kernel for MembranePotentialDecoder.

Computes the final state of the leaky-integrator scan
    mem_t = mem_{t-1} * decay + spike_t,  mem_{-1} = 0
which closed-form is the weighted reduction
    out[b, n] = sum_t decay^(T-1-t) * spikes[b, t, n].

Strategy: data-parallel over batch B across 8 NeuronCores (4 batches each).
Per core, each batch's (T=512, N=2048) fp32 slab is DMA'd as one contiguous
4 MiB transfer into an SBUF tile [128 partitions, 16384], partition p holding
time rows 4p..4p+3.  The weighted reduction over T runs on the TensorEngine:
for each of the 4 row-offsets r, a matmul with stationary weight column
w[4p+r] contracts the 128 partitions, accumulating the 4 row-offsets into
PSUM.  float32r (single-pass FP22-truncated fp32 matmul) keeps the PE at
1 cycle/row so the kernel stays DMA-bound (~47 us/core roofline).
"""

import sys

import numpy as np

if "/opt/trn_rl_repo" not in sys.path:
    sys.path.insert(0, "/opt/trn_rl_repo")

import concourse.bass as bass  # noqa: F401  (engine namespaces live on nc)
import concourse.tile as tile
from concourse import bacc, mybir
from concourse.bass_utils import run_bass_kernel_spmd

TAU = 10.0
B, T, N = 32, 512, 2048
NCORES = 8
B_LOC = B // NCORES          # 4 batches per core
ROWS_PER_PART = T // 128     # 4 time rows folded into each partition
NCHUNK = N // 512            # 4 matmul column chunks (PSUM bank = 512 fp32)

# Set by test harness to enable NTFF profiling; results stashed here.
PROFILE = False
LAST_RESULTS = None


def _weights() -> np.ndarray:
    """w_in[p, r] = decay^(T-1 - (4p + r)) as fp32, matching the fp32 scan."""
    decay = np.float64(np.exp(np.float32(-1.0 / TAU), dtype=np.float32))
    t = (np.arange(128)[:, None] * ROWS_PER_PART + np.arange(ROWS_PER_PART)[None, :])
    return (decay ** (T - 1 - t)).astype(np.float32)


def _build_program():
    nc = bacc.Bacc(
        "TRN2",
        target_bir_lowering=False,
        debug=False,
        enable_asserts=True,
        num_devices=NCORES,
    )
    f32 = mybir.dt.float32
    f32r = mybir.dt.float32r

    x = nc.dram_tensor("spikes", [B_LOC, T, N], f32r, kind="ExternalInput").ap()
    w = nc.dram_tensor("w", [128, ROWS_PER_PART], f32r, kind="ExternalInput").ap()
    out = nc.dram_tensor("out", [B_LOC, N], f32, kind="ExternalOutput").ap()

    with tile.TileContext(nc) as tc:
        with (
            tc.tile_pool(name="wpool", bufs=1) as wpool,
            tc.tile_pool(name="xpool", bufs=2) as xpool,
            tc.tile_pool(name="opool", bufs=2) as opool,
            tc.tile_pool(name="ppool", bufs=8, space="PSUM") as ppool,
        ):
            wt = wpool.tile([128, ROWS_PER_PART], f32r)
            nc.sync.dma_start(wt[:], w[:])

            for b in range(B_LOC):
                xt = xpool.tile([128, ROWS_PER_PART * N], f32r)
                nc.sync.dma_start(
                    xt[:], x[b].rearrange("(p r) n -> p (r n)", p=128)
                )
                ot = opool.tile([1, N], f32)
                for c in range(NCHUNK):
                    ps = ppool.tile([1, 512], f32)
                    for r in range(ROWS_PER_PART):
                        nc.tensor.matmul(
                            ps[:],
                            wt[:, r : r + 1],
                            xt[:, r * N + c * 512 : r * N + (c + 1) * 512],
                            start=(r == 0),
                            stop=(r == ROWS_PER_PART - 1),
                        )
                    # spread PSUM->SBUF copies across DVE and ACT
                    dst = ot[:, c * 512 : (c + 1) * 512]
                    if c % 2 == 0:
                        nc.vector.tensor_copy(dst, ps[:])
                    else:
                        nc.scalar.copy(dst, ps[:])
                nc.sync.dma_start(out[b : b + 1, :], ot[:])

    nc.compile()
    return nc


def kernel(spikes: np.ndarray) -> np.ndarray:
    global LAST_RESULTS
    spikes = np.ascontiguousarray(np.asarray(spikes, dtype=np.float32))
    assert spikes.shape == (B, T, N), spikes.shape

    nc = _build_program()
    w_in = _weights()
    in_maps = [
        {"spikes": spikes[i * B_LOC : (i + 1) * B_LOC], "w": w_in}
        for i in range(NCORES)
    ]
    res = run_bass_kernel_spmd(nc, in_maps, list(range(NCORES)), trace=PROFILE)
    LAST_RESULTS = res
    return np.concatenate([res.results[i]["out"] for i in range(NCORES)], axis=0)


# revision 11
# speedup vs baseline: 1.0894x; 1.0894x over previous
"""Trainium2 Bass kernel for MembranePotentialDecoder.

Computes the final state of the leaky-integrator scan
    mem_t = mem_{t-1} * decay + spike_t,  mem_{-1} = 0
which closed-form is the weighted reduction
    out[b, n] = sum_t decay^(T-1-t) * spikes[b, t, n].

Strategy: data-parallel over batch B across 8 NeuronCores (4 batches each).
Per core, each batch's (T=512, N=2048) fp32 slab is DMA'd as one contiguous
4 MiB transfer into an SBUF tile [128 partitions, 16384], partition p holding
time rows 4p..4p+3.  The weighted reduction over T runs on the TensorEngine:
for each of the 4 row-offsets r, a matmul with stationary weight column
w[4p+r] contracts the 128 partitions, accumulating the 4 row-offsets into
PSUM.  float32r (single-pass FP22-truncated fp32 matmul) keeps the PE at
1 cycle/row so the kernel stays DMA-bound (~47 us/core roofline).
"""

import sys

import numpy as np

if "/opt/trn_rl_repo" not in sys.path:
    sys.path.insert(0, "/opt/trn_rl_repo")

import concourse.bass as bass  # noqa: F401  (engine namespaces live on nc)
import concourse.tile as tile
from concourse import bacc, mybir
from concourse.bass_utils import run_bass_kernel_spmd

TAU = 10.0
B, T, N = 32, 512, 2048
NCORES = 8
B_LOC = B // NCORES          # 4 batches per core
ROWS_PER_PART = T // 128     # 4 time rows folded into each partition
NCHUNK = N // 512            # 4 matmul column chunks (PSUM bank = 512 fp32)

# Set by test harness to enable NTFF profiling; results stashed here.
PROFILE = False
LAST_RESULTS = None


def _weights() -> np.ndarray:
    """w_in[p, j] = decay^(T-1 - (128j + p)) as fp32: column j is the weight
    vector for t-tile j (rows 128j..128j+127 of the scan)."""
    decay = np.float64(np.exp(np.float32(-1.0 / TAU), dtype=np.float32))
    t = np.arange(128)[:, None] + 128 * np.arange(ROWS_PER_PART)[None, :]
    return (decay ** (T - 1 - t)).astype(np.float32)


def _build_program():
    nc = bacc.Bacc(
        "TRN2",
        target_bir_lowering=False,
        debug=False,
        enable_asserts=False,
        num_devices=NCORES,
    )
    f32 = mybir.dt.float32
    f32r = mybir.dt.float32r

    x = nc.dram_tensor("spikes", [B_LOC, T, N], f32r, kind="ExternalInput").ap()
    w = nc.dram_tensor("w", [128, ROWS_PER_PART], f32r, kind="ExternalInput").ap()
    out = nc.dram_tensor("out", [B_LOC, N], f32, kind="ExternalOutput").ap()

    with tile.TileContext(nc) as tc:
        with (
            tc.tile_pool(name="wpool", bufs=1) as wpool,
            tc.tile_pool(name="xpool", bufs=10) as xpool,
            tc.tile_pool(name="opool", bufs=2) as opool,
            tc.tile_pool(name="ppool", bufs=8, space="PSUM") as ppool,
        ):
            # tiny weight load goes via SWDGE so it never blocks the sync
            # HWDGE ring that streams the 1 MiB input tiles
            wt = wpool.tile([128, ROWS_PER_PART], f32r)
            nc.gpsimd.dma_start(wt[:], w[:])

            # x viewed as t-tiles: [b, j, p, n] with t = 128j + p
            xv = x.rearrange("b (j p) n -> b j p n", p=128)

            for b in range(B_LOC):
                pss = []
                for j in range(ROWS_PER_PART):
                    xt = xpool.tile([128, N], f32r)
                    nc.sync.dma_start(xt[:], xv[b, j])
                    for c in range(NCHUNK):
                        if j == 0:
                            pss.append(
                                ppool.tile([1, 512], f32, name=f"ps{b}_{c}", tag="ps")
                            )
                        nc.tensor.matmul(
                            pss[c][:],
                            wt[:, j : j + 1],
                            xt[:, c * 512 : (c + 1) * 512],
                            start=(j == 0),
                            stop=(j == ROWS_PER_PART - 1),
                        )
                ot = opool.tile([1, N], f32)
                for c in range(NCHUNK):
                    # spread PSUM->SBUF copies across DVE and ACT
                    dst = ot[:, c * 512 : (c + 1) * 512]
                    if c % 2 == 0:
                        nc.vector.tensor_copy(dst, pss[c][:])
                    else:
                        nc.scalar.copy(dst, pss[c][:])
                # out DMA on the ACT HWDGE ring: the sync ring must stay a
                # pure back-to-back input stream (a sem-waiting out DMA on
                # it would stall all loads queued behind it)
                nc.scalar.dma_start(out[b : b + 1, :], ot[:])

    nc.compile()
    return nc


def kernel(spikes: np.ndarray) -> np.ndarray:
    global LAST_RESULTS
    spikes = np.ascontiguousarray(np.asarray(spikes, dtype=np.float32))
    assert spikes.shape == (B, T, N), spikes.shape

    nc = _build_program()
    w_in = _weights()
    in_maps = [
        {"spikes": spikes[i * B_LOC : (i + 1) * B_LOC], "w": w_in}
        for i in range(NCORES)
    ]
    res = run_bass_kernel_spmd(nc, in_maps, list(range(NCORES)), trace=PROFILE)
    LAST_RESULTS = res
    return np.concatenate([res.results[i]["out"] for i in range(NCORES)], axis=0)
